# revision 14
# baseline (speedup 1.0000x reference)
"""Causal attention (single head) on 8 Trainium2 NeuronCores.

Problem: x[4096,1024], Wq/Wk/Wv[1024,1024] (torch Linear layout, applied as
x @ W.T); out = renormalized-causal-softmax(Q K^T / 32) @ V, fp32, [4096,1024].

Distribution (hardcoded for S=4096, D=1024, 8 cores):
  - Q rows are sharded STRIDED: core c owns rows c::8.  With 128-row q-tiles,
    tile qt of every core spans global rows [1024qt, 1024qt+1024), so every
    core has identical causal trip counts (SPMD: one program, data-only
    variation).  The causal mask is shipped as a per-core input tensor.
  - K/V rows are sharded CONTIGUOUS: core c computes K^T,V for rows
    [512c, 512c+512) in fp8e3 (e3m4: 4-bit mantissa, K/V values are well
    inside its range, and it HALVES the AllGather bytes vs bf16/e4m3 --
    the serial CC stream is the scarce resource).  Measured: each collective
    op costs a ~21us fixed floor + ~8us/MB wire, so exactly TWO collectives
    are used: AG(KT, 2MB out) then AG(V, 4MB out); Q^T projection and all
    S^T/exp work run under AG(V).
  - Attention is computed TRANSPOSED: S^T[k, q] = K^T-chunk.T @ Q^T with the
    128-key chunk as the stationary operand.  exp(S^T) then IS P^T, the
    exact lhsT layout the P@V matmuls need -- no PE transposes, no PSUM
    round-trips.  Causality: key chunk kc is needed only by q >= 128*(kc//8),
    so S^T rows shrink as kc grows; the intra-chunk diagonal mask (first 128
    q columns) is an additive -30000 shipped as data (8 distinct patterns).
  - Denominators: P@V runs with P^T chunks stationary; a third 1-column
    matmul against a ones vector rides on each stationary load and
    accumulates sum_k P^T[k, q] in PSUM -- the softmax denominator for free.
    The reference's "softmax -> tril -> renormalize" is algebraically
    masked-exp / masked-sum (the dense softmax denominator cancels), and
    scores/32 are within +-4 so exp needs no max-subtraction.
  - All matmuls bf16/fp8 (fp32 matmul is 4x slower); PSUM accum is fp32.
"""

import numpy as np
import ml_dtypes

S, D, NC_N = 4096, 1024, 8
QROWS = S // NC_N            # 512 q rows per core
KVROWS = S // NC_N           # 512 kv rows per core
NQT = QROWS // 128           # 4 q-tiles of 128 rows per core
DC = D // 128                # 8 contraction chunks
NKC = S // 128               # 32 global key chunks
BF16 = ml_dtypes.bfloat16

_CACHE = {}


def _build():
    import concourse.bass as bass
    import concourse.mybir as mybir
    import concourse.tile as tile
    from concourse import bacc

    fp32 = mybir.dt.float32
    bf16 = mybir.dt.bfloat16
    fp8 = mybir.dt.float8e3

    nc = bacc.Bacc("TRN2", target_bir_lowering=False, debug=False,
                   num_devices=NC_N, enable_asserts=False)

    xt_q = nc.dram_tensor("xt_q", [D, QROWS], bf16, kind="ExternalInput").ap()
    xt_kv = nc.dram_tensor("xt_kv", [D, KVROWS], bf16, kind="ExternalInput").ap()
    wqt = nc.dram_tensor("wqt", [D, D], bf16, kind="ExternalInput").ap()
    wkt = nc.dram_tensor("wkt", [D, D], bf16, kind="ExternalInput").ap()
    wvt = nc.dram_tensor("wvt", [D, D], bf16, kind="ExternalInput").ap()
    maskT = nc.dram_tensor("maskT", [128, 16], fp32, kind="ExternalInput").ap()
    out = nc.dram_tensor("out", [QROWS, D], fp32, kind="ExternalOutput").ap()

    rg = [list(range(NC_N))]
    inv_sqrt_d = 1.0 / np.sqrt(np.float32(D))

    with tile.TileContext(nc) as tc:
        with (
            tc.tile_pool(name="dram", bufs=1, space="DRAM") as dram,
            tc.tile_pool(name="const", bufs=1) as cpool,
            tc.tile_pool(name="kvres", bufs=1) as kvpool,
            tc.tile_pool(name="stats", bufs=4) as stpool,
        ):
            warm_cc_in = dram.tile([128, 16], fp8, name="warm_cc_in")
            warm_cc_out = dram.tile([NC_N, 128, 16], fp8, name="warm_cc_out",
                                    addr_space="Shared")
            kt_cc_in = dram.tile([D, KVROWS], fp8, name="kt_cc_in")
            v_cc_in = dram.tile([KVROWS, D], fp8, name="v_cc_in")
            kt_cc_out = dram.tile([NC_N, D, KVROWS], fp8, name="kt_cc_out",
                                  addr_space="Shared")
            v_cc_out = dram.tile([NC_N, KVROWS, D], fp8, name="v_cc_out",
                                  addr_space="Shared")

            ones_sb = cpool.tile([128, 1], bf16, name="ones_sb")
            nc.any.memset(ones_sb[:], 1.0)
            mask_sb = cpool.tile([128, 16], fp32, name="mask_sb")
            nc.scalar.dma_start(mask_sb[:], maskT[:])

            # The CC stream pays a large one-time per-execution arming cost
            # (~50us) before its first bytes move.  Fire a tiny dummy
            # AllGather immediately so that cost overlaps the projections
            # instead of delaying AG(KT).
            warm_sb = cpool.tile([128, 16], fp8, name="warm_sb")
            nc.vector.memset(warm_sb[:], 0.0)
            nc.sync.dma_start(warm_cc_in[:], warm_sb[:])
            nc.gpsimd.collective_compute(
                "AllGather", mybir.AluOpType.bypass, replica_groups=rg,
                ins=[warm_cc_in[:]], outs=[warm_cc_out[:]])

            # gathered K^T: ktf[r][p, dc*512+j] = K[512r+j, 128dc+p]
            ktf = [kvpool.tile([128, DC * 512], fp8, name=f"ktf{r}")
                   for r in range(NC_N)]
            # gathered V halves: vfA[dh][r][p, sl*512+j] = V[512r+128sl+p,
            # 512dh+j] for sl in {0,1}; vfB the same for sl in {2,3}.
            vf = [[kvpool.tile([128, 4 * 512], fp8, name=f"vf{dh}_{r}")
                   for r in range(NC_N)] for dh in range(2)]
            # Q^T resident: qt_sb[p, dc*512 + q] = Q[q, 128dc+p]
            qt_sb = kvpool.tile([128, DC * 512], bf16, name="qt_sb")
            # P^T resident: ptall[p, kc*512 + q] = P[q, 128kc+p]
            ptall = kvpool.tile([128, NKC * 512], bf16, name="ptall")

            # ---------------- phase 1: projections + gathers ----------------
            with (
                tc.tile_pool(name="wpool", bufs=12) as wpool,
                tc.tile_pool(name="xpool", bufs=1) as xpool,
                tc.tile_pool(name="loc", bufs=4) as locpool,
                tc.tile_pool(name="ppsum", bufs=3, space="PSUM") as ppsum,
            ):
                # PE warmup: the HAM clock gate holds the PE at 1.2 GHz until
                # it has been busy ~3.4us.  Burn dummy matmuls during the
                # initial weight DMA so the K projection runs at 2.4 GHz.
                wrm = cpool.tile([128, 512], bf16, name="wrm")
                nc.vector.memset(wrm[:], 0.0)
                wps = ppsum.tile([128, 512], fp32, tag="warm")
                for i in range(24):
                    nc.tensor.matmul(wps[:], wrm[:, 0:128], wrm[:],
                                     start=(i == 0), stop=(i == 23))

                # K-projection inputs first (critical path to the CC stream)
                wk, xkv = [], []
                for dc in range(DC):
                    tw = wpool.tile([128, D], bf16, name=f"wk{dc}", tag="w")
                    nc.scalar.dma_start(tw[:], wkt[dc * 128:(dc + 1) * 128, :])
                    wk.append(tw)
                    tx = xpool.tile([128, KVROWS], bf16, name=f"xkv{dc}")
                    nc.scalar.dma_start(tx[:], xt_kv[dc * 128:(dc + 1) * 128, :])
                    xkv.append(tx)

                # K^T_local[d, s] = (Wk @ x_kv^T): lhsT = Wk^T chunk, rhs = x_kv^T
                for po in range(DC):
                    ps = ppsum.tile([128, 512], fp32, tag="pp")
                    for dc in range(DC):
                        nc.tensor.matmul(ps[:], wk[dc][:, po * 128:(po + 1) * 128],
                                         xkv[dc][:],
                                         start=(dc == 0), stop=(dc == DC - 1))
                    loc = locpool.tile([128, 512], fp8, tag="lock")
                    nc.vector.tensor_copy(loc[:], ps[:])
                    nc.sync.dma_start(kt_cc_in[po * 128:(po + 1) * 128, :], loc[:])

                nc.gpsimd.collective_compute(
                    "AllGather", mybir.AluOpType.bypass, replica_groups=rg,
                    ins=[kt_cc_in[:]], outs=[kt_cc_out[:]])

                # V_local[s, d] = x_kv @ Wv^T: lhsT = x_kv^T chunk, rhs = Wv^T
                wv = []
                for dc in range(DC):
                    tw = wpool.tile([128, D], bf16, name=f"wv{dc}", tag="w")
                    nc.scalar.dma_start(tw[:], wvt[dc * 128:(dc + 1) * 128, :])
                    wv.append(tw)
                for st in range(4):
                    for dh in range(2):
                        ps = ppsum.tile([128, 512], fp32, tag="pp")
                        for dc in range(DC):
                            nc.tensor.matmul(
                                ps[:], xkv[dc][:, st * 128:(st + 1) * 128],
                                wv[dc][:, dh * 512:(dh + 1) * 512],
                                start=(dc == 0), stop=(dc == DC - 1))
                        loc = locpool.tile([128, 512], fp8, tag="locv")
                        nc.vector.tensor_copy(loc[:], ps[:])
                        nc.sync.dma_start(
                            v_cc_in[st * 128:(st + 1) * 128,
                                    dh * 512:(dh + 1) * 512], loc[:])
                nc.gpsimd.collective_compute(
                    "AllGather", mybir.AluOpType.bypass, replica_groups=rg,
                    ins=[v_cc_in[:]], outs=[v_cc_out[:]])

                # Q^T[d, q]: lhsT = Wq^T chunk, rhs = x_q^T  -> straight to SBUF
                wq, xq = [], []
                for dc in range(DC):
                    tw = wpool.tile([128, D], bf16, name=f"wq{dc}", tag="w")
                    nc.scalar.dma_start(tw[:], wqt[dc * 128:(dc + 1) * 128, :])
                    wq.append(tw)
                    tx = xpool.tile([128, QROWS], bf16, name=f"xq{dc}")
                    nc.scalar.dma_start(tx[:], xt_q[dc * 128:(dc + 1) * 128, :])
                    xq.append(tx)
                for po in range(DC):
                    ps = ppsum.tile([128, 512], fp32, tag="pp")
                    for dc in range(DC):
                        nc.tensor.matmul(ps[:], wq[dc][:, po * 128:(po + 1) * 128],
                                         xq[dc][:],
                                         start=(dc == 0), stop=(dc == DC - 1))
                    nc.vector.tensor_copy(qt_sb[:, po * 512:(po + 1) * 512], ps[:])

            # ---------------- phase 2: pull gathered K/V into SBUF ----------
            for r in range(NC_N):
                eng = nc.sync if r % 2 == 0 else nc.scalar
                eng.dma_start(
                    ktf[r][:].rearrange("p (a j) -> p a j", a=DC),
                    kt_cc_out[r].rearrange("(a p) j -> p a j", p=128))
            for r in range(NC_N):
                for dh in range(2):
                    eng = nc.sync if (2 * r + dh) % 2 == 0 else nc.scalar
                    eng.dma_start(
                        vf[dh][r][:].rearrange("p (a j) -> p a j", a=4),
                        v_cc_out[r].rearrange("(a p) (b j) -> p a b j",
                                              p=128, b=2)[:, :, dh, :])

            # ---------------- phase 3a: S^T + exp -> P^T ----------------
            # S^T[k, q] per 128-key chunk kc: stationary = K^T chunk, moving =
            # Q^T.  Causal: q row c+8q sees key 128kc+j iff c+8q >= 128kc+j,
            # so only q >= 16kc is reachable (uniformly over c), and within
            # that only the first 16 q columns are partially masked -- by ONE
            # shared [128,16] pattern (keep iff c+8u >= j).  exp() writes P^T
            # straight into ptall (absolute q addressing); the causally-dead
            # strip [128*(kc//8), 16kc) that P@V still touches is zeroed.
            with tc.tile_pool(name="spsum", bufs=6, space="PSUM") as spsum:
                for kc in range(NKC):
                    qt0 = kc // 8
                    qoff = 16 * kc
                    w = 512 - qoff
                    r, kci = kc // 4, kc % 4
                    if qoff > 128 * qt0:
                        nc.vector.memset(
                            ptall[:, kc * 512 + 128 * qt0:kc * 512 + qoff], 0.0)
                    psT = spsum.tile([128, 512], fp32, tag="s")
                    for dc in range(DC):
                        nc.tensor.matmul(
                            psT[:, :w],
                            ktf[r][:, dc * 512 + kci * 128:
                                   dc * 512 + (kci + 1) * 128],
                            qt_sb[:, dc * 512 + qoff:(dc + 1) * 512],
                            start=(dc == 0), stop=(dc == DC - 1))
                    nc.vector.tensor_add(psT[:, 0:16], psT[:, 0:16],
                                         mask_sb[:])
                    nc.scalar.activation(
                        ptall[:, kc * 512 + qoff:(kc + 1) * 512], psT[:, :w],
                        mybir.ActivationFunctionType.Exp,
                        bias=0.0, scale=float(inv_sqrt_d))

            # ---------------- phase 3b: P@V + denominators ----------------
            with (
                tc.tile_pool(name="obuf", bufs=2) as opool,
                tc.tile_pool(name="opsum", bufs=2, space="PSUM") as opsum,
            ):
                for qt in range(NQT):
                    nkc = 8 * (qt + 1)
                    pso = [opsum.tile([128, 512], fp32, tag=f"po{dh}",
                                      name=f"pso{qt}_{dh}") for dh in range(2)]
                    denp = opsum.tile([128, 1], fp32, tag="den",
                                      name=f"den{qt}")
                    for kc in range(nkc):
                        r, sl = kc // 4, kc % 4
                        lhs = ptall[:, kc * 512 + qt * 128:
                                    kc * 512 + (qt + 1) * 128]
                        st = (kc == 0)
                        sp = (kc == nkc - 1)
                        # lhsT (P^T chunk) shared by the three rhs -> one
                        # stationary load serves d-half 0, d-half 1, denom
                        nc.tensor.matmul(pso[0][:], lhs,
                                         vf[0][r][:, sl * 512:(sl + 1) * 512],
                                         start=st, stop=sp)
                        nc.tensor.matmul(pso[1][:], lhs,
                                         vf[1][r][:, sl * 512:(sl + 1) * 512],
                                         start=st, stop=sp)
                        nc.tensor.matmul(denp[:], lhs, ones_sb[:],
                                         start=st, stop=sp)
                    recip = stpool.tile([128, 1], fp32, tag="recip")
                    nc.vector.reciprocal(recip[:], denp[:])
                    o_sb = opool.tile([128, D], fp32, tag="o")
                    for dh in range(2):
                        nc.vector.tensor_scalar_mul(
                            o_sb[:, dh * 512:(dh + 1) * 512], pso[dh][:],
                            recip[:])
                    nc.sync.dma_start(out[qt * 128:(qt + 1) * 128, :], o_sb[:])

    nc.compile()
    return nc


def _get_nc():
    if "nc" not in _CACHE:
        _CACHE["nc"] = _build()
    return _CACHE["nc"]


def make_in_maps(x, Wq, Wk, Wv):
    x_bf = np.ascontiguousarray(x).astype(BF16)
    wqt = np.ascontiguousarray(Wq.astype(BF16).T)
    wkt = np.ascontiguousarray(Wk.astype(BF16).T)
    wvt = np.ascontiguousarray(Wv.astype(BF16).T)
    in_maps = []
    j = np.arange(128)[:, None]                     # key-within-chunk
    u = np.arange(16)[None, :]                      # q - 16*kc
    for c in range(NC_N):
        # q row (global) = c + 8*(16*kc + u); key = 128*kc + j
        # keep iff c + 8*u >= j  (kc-independent)
        maskT = np.where(c + 8 * u >= j, 0.0, -30000.0).astype(np.float32)
        xt_q = np.ascontiguousarray(x_bf[c::NC_N].T)
        xt_kv = np.ascontiguousarray(x_bf[c * KVROWS:(c + 1) * KVROWS].T)
        in_maps.append({"xt_q": xt_q, "xt_kv": xt_kv, "wqt": wqt,
                        "wkt": wkt, "wvt": wvt, "maskT": maskT})
    return in_maps


def run(in_maps, trace=False, tmpdir=None, trace_cores=None):
    from concourse.bass_utils import run_bass_kernel_spmd
    nc = _get_nc()
    return run_bass_kernel_spmd(nc, in_maps, core_ids=list(range(NC_N)),
                                trace=trace, tmpdir=tmpdir,
                                trace_cores=trace_cores)


def kernel(x, Wq, Wk, Wv):
    res = run(make_in_maps(np.asarray(x), np.asarray(Wq),
                           np.asarray(Wk), np.asarray(Wv)))
    full = np.empty((S, D), np.float32)
    for c in range(NC_N):
        full[c::NC_N] = res.results[c]["out"]
    return full


# revision 17
# speedup vs baseline: 1.0048x; 1.0048x over previous
"""Causal attention (single head) on 8 Trainium2 NeuronCores.

Problem: x[4096,1024], Wq/Wk/Wv[1024,1024] (torch Linear layout, applied as
x @ W.T); out = renormalized-causal-softmax(Q K^T / 32) @ V, fp32, [4096,1024].

Distribution (hardcoded for S=4096, D=1024, 8 cores):
  - Q rows are sharded STRIDED: core c owns rows c::8, so every core has
    identical causal trip counts (SPMD: one program, data-only variation).
  - K/V rows are sharded CONTIGUOUS: core c computes K^T,V for rows
    [512c, 512c+512) in fp8e3 (e3m4: 4-bit mantissa; K/V values sit well
    inside its range, and it halves the exchange bytes vs bf16).
  - Measured CC behavior on this runtime: the collective stream moves no
    bytes until ~75us after EXECUTION start (per-exec firmware arming,
    independent of trigger time or op count), then streams at ~60-100GB/s
    wire with small per-op overhead.  So the kernel splits the exchange into
    FOUR AllGathers -- K^T keys 0-255 / 256-511 of each shard, V rows
    0-255 / 256-511 -- and orders compute so each piece is consumed as it
    lands: S^T on the first K half runs under the later AGs, P@V on the
    first V half runs under nothing it doesn't need.
  - Attention is computed TRANSPOSED: S^T[k, q] = K^T-chunk.T @ Q^T with the
    128-key chunk stationary.  exp(S^T) IS P^T, exactly the lhsT layout the
    P@V matmuls need -- no PE transposes.  Causality: key chunk kc is needed
    only by q >= 16*kc (exact, uniformly over cores); within that only the
    first 16 q columns are partially masked, by ONE shared [128,16] additive
    -30000 pattern (keep iff c+8u >= j, core-dependent data).  The
    causally-dead strip [128*(kc//8), 16*kc) that P@V still touches is
    zeroed.  "softmax -> tril -> renormalize" == masked-exp / masked-sum,
    and scores/32 are within +-4 so exp needs no max-subtraction.
  - Denominators: P@V runs with P^T chunks stationary; a third 1-column
    matmul against ones rides on each stationary load and accumulates
    sum_k P^T[k, q] in PSUM -- the softmax denominator for free.
  - All matmuls bf16/fp8 mixed (fp32 matmul is 4x slower); PSUM accum fp32.
"""

import numpy as np
import ml_dtypes

S, D, NC_N = 4096, 1024, 8
QROWS = S // NC_N            # 512 q rows per core
KVROWS = S // NC_N           # 512 kv rows per core
NQT = QROWS // 128           # 4 q-tiles of 128 rows per core
DC = D // 128                # 8 contraction chunks
NKC = S // 128               # 32 global key chunks
BF16 = ml_dtypes.bfloat16

_CACHE = {}


def _build():
    import concourse.bass as bass
    import concourse.mybir as mybir
    import concourse.tile as tile
    from concourse import bacc

    fp32 = mybir.dt.float32
    bf16 = mybir.dt.bfloat16
    fp8 = mybir.dt.float8e3

    nc = bacc.Bacc("TRN2", target_bir_lowering=False, debug=False,
                   num_devices=NC_N, enable_asserts=False)

    xt_q = nc.dram_tensor("xt_q", [D, QROWS], bf16, kind="ExternalInput").ap()
    xt_kv = nc.dram_tensor("xt_kv", [D, KVROWS], bf16, kind="ExternalInput").ap()
    wqt = nc.dram_tensor("wqt", [D, D], bf16, kind="ExternalInput").ap()
    wkt = nc.dram_tensor("wkt", [D, D], bf16, kind="ExternalInput").ap()
    wvt = nc.dram_tensor("wvt", [D, D], bf16, kind="ExternalInput").ap()
    maskT = nc.dram_tensor("maskT", [128, 16], fp32, kind="ExternalInput").ap()
    out = nc.dram_tensor("out", [QROWS, D], fp32, kind="ExternalOutput").ap()

    rg = [list(range(NC_N))]
    inv_sqrt_d = 1.0 / np.sqrt(np.float32(D))

    with tile.TileContext(nc) as tc:
        with (
            tc.tile_pool(name="dram", bufs=1, space="DRAM") as dram,
            tc.tile_pool(name="const", bufs=1) as cpool,
            tc.tile_pool(name="kvres", bufs=1) as kvpool,
            tc.tile_pool(name="stats", bufs=4) as stpool,
        ):
            kta_in = dram.tile([D, 256], fp8, name="kta_in")
            ktb_in = dram.tile([D, 256], fp8, name="ktb_in")
            va_in = dram.tile([256, D], fp8, name="va_in")
            vb_in = dram.tile([256, D], fp8, name="vb_in")
            kta_out = dram.tile([NC_N, D, 256], fp8, name="kta_out",
                                addr_space="Shared")
            ktb_out = dram.tile([NC_N, D, 256], fp8, name="ktb_out",
                                addr_space="Shared")
            va_out = dram.tile([NC_N, 256, D], fp8, name="va_out",
                               addr_space="Shared")
            vb_out = dram.tile([NC_N, 256, D], fp8, name="vb_out",
                               addr_space="Shared")

            ones_sb = cpool.tile([128, 1], bf16, name="ones_sb")
            nc.any.memset(ones_sb[:], 1.0)
            mask_sb = cpool.tile([128, 16], fp32, name="mask_sb")
            nc.scalar.dma_start(mask_sb[:], maskT[:])

            # gathered K^T halves: ktfa[r][p, dc*256+j] = K[512r+j, 128dc+p]
            # for j in [0,256); ktfb the same for j-256 in [256,512).
            ktfa = [kvpool.tile([128, DC * 256], fp8, name=f"ktfa{r}")
                    for r in range(NC_N)]
            ktfb = [kvpool.tile([128, DC * 256], fp8, name=f"ktfb{r}")
                    for r in range(NC_N)]
            # gathered V halves: vfA[dh][r][p, sl*512+j] = V[512r+128sl+p,
            # 512dh+j] for sl in {0,1}; vfB the same for sl in {2,3}.
            vfA = [[kvpool.tile([128, 2 * 512], fp8, name=f"vfA{dh}_{r}")
                    for r in range(NC_N)] for dh in range(2)]
            vfB = [[kvpool.tile([128, 2 * 512], fp8, name=f"vfB{dh}_{r}")
                    for r in range(NC_N)] for dh in range(2)]
            # Q^T resident: qt_sb[p, dc*512 + q] = Q[q, 128dc+p]
            qt_sb = kvpool.tile([128, DC * 512], bf16, name="qt_sb")
            # P^T resident: ptall[p, kc*512 + q] = P[q, 128kc+p]
            ptall = kvpool.tile([128, NKC * 512], bf16, name="ptall")

            # ---------------- phase 1: projections + gathers ----------------
            with (
                tc.tile_pool(name="wpool", bufs=12) as wpool,
                tc.tile_pool(name="xpool", bufs=1) as xpool,
                tc.tile_pool(name="loc", bufs=4) as locpool,
                tc.tile_pool(name="ppsum", bufs=3, space="PSUM") as ppsum,
            ):
                # PE warmup: the HAM clock gate holds the PE at 1.2 GHz until
                # it has been busy ~3.4us.  Burn dummy matmuls during the
                # initial weight DMA so the K projection runs at 2.4 GHz.
                wrm = cpool.tile([128, 512], bf16, name="wrm")
                nc.vector.memset(wrm[:], 0.0)
                wps = ppsum.tile([128, 512], fp32, tag="warm")
                for i in range(24):
                    nc.tensor.matmul(wps[:], wrm[:, 0:128], wrm[:],
                                     start=(i == 0), stop=(i == 23))

                # K-projection inputs first (critical path to the CC stream)
                wk, xkv = [], []
                for dc in range(DC):
                    tw = wpool.tile([128, D], bf16, name=f"wk{dc}", tag="w")
                    nc.scalar.dma_start(tw[:], wkt[dc * 128:(dc + 1) * 128, :])
                    wk.append(tw)
                    tx = xpool.tile([128, KVROWS], bf16, name=f"xkv{dc}")
                    nc.scalar.dma_start(tx[:], xt_kv[dc * 128:(dc + 1) * 128, :])
                    xkv.append(tx)

                # K^T_local[d, s] = (Wk @ x_kv^T), computed key-half-major so
                # AG(KT keys 0-255) can fire at half-projection time.
                for kh, (kin, _) in enumerate(((kta_in, kta_out),
                                               (ktb_in, ktb_out))):
                    for po in range(DC):
                        ps = ppsum.tile([128, 512], fp32, tag="pp")
                        for dc in range(DC):
                            nc.tensor.matmul(
                                ps[:, 0:256], wk[dc][:, po * 128:(po + 1) * 128],
                                xkv[dc][:, kh * 256:(kh + 1) * 256],
                                start=(dc == 0), stop=(dc == DC - 1))
                        loc = locpool.tile([128, 256], fp8, tag="lock")
                        nc.vector.tensor_copy(loc[:], ps[:, 0:256])
                        nc.sync.dma_start(kin[po * 128:(po + 1) * 128, :],
                                          loc[:])
                    nc.gpsimd.collective_compute(
                        "AllGather", mybir.AluOpType.bypass, replica_groups=rg,
                        ins=[kin[:]],
                        outs=[(kta_out if kh == 0 else ktb_out)[:]])

                # V_local[s, d] = x_kv @ Wv^T: lhsT = x_kv^T chunk, rhs = Wv^T
                wv = []
                for dc in range(DC):
                    tw = wpool.tile([128, D], bf16, name=f"wv{dc}", tag="w")
                    nc.scalar.dma_start(tw[:], wvt[dc * 128:(dc + 1) * 128, :])
                    wv.append(tw)
                for st in range(4):
                    for dh in range(2):
                        ps = ppsum.tile([128, 512], fp32, tag="pp")
                        for dc in range(DC):
                            nc.tensor.matmul(
                                ps[:], xkv[dc][:, st * 128:(st + 1) * 128],
                                wv[dc][:, dh * 512:(dh + 1) * 512],
                                start=(dc == 0), stop=(dc == DC - 1))
                        loc = locpool.tile([128, 512], fp8, tag="locv")
                        nc.vector.tensor_copy(loc[:], ps[:])
                        tgt = va_in if st < 2 else vb_in
                        nc.sync.dma_start(
                            tgt[(st % 2) * 128:(st % 2) * 128 + 128,
                                dh * 512:(dh + 1) * 512], loc[:])
                    if st == 1:
                        nc.gpsimd.collective_compute(
                            "AllGather", mybir.AluOpType.bypass,
                            replica_groups=rg,
                            ins=[va_in[:]], outs=[va_out[:]])
                nc.gpsimd.collective_compute(
                    "AllGather", mybir.AluOpType.bypass, replica_groups=rg,
                    ins=[vb_in[:]], outs=[vb_out[:]])

                # Q^T[d, q]: lhsT = Wq^T chunk, rhs = x_q^T  -> straight to SBUF
                wq, xq = [], []
                for dc in range(DC):
                    tw = wpool.tile([128, D], bf16, name=f"wq{dc}", tag="w")
                    nc.scalar.dma_start(tw[:], wqt[dc * 128:(dc + 1) * 128, :])
                    wq.append(tw)
                    tx = xpool.tile([128, QROWS], bf16, name=f"xq{dc}")
                    nc.scalar.dma_start(tx[:], xt_q[dc * 128:(dc + 1) * 128, :])
                    xq.append(tx)
                for po in range(DC):
                    ps = ppsum.tile([128, 512], fp32, tag="pp")
                    for dc in range(DC):
                        nc.tensor.matmul(ps[:], wq[dc][:, po * 128:(po + 1) * 128],
                                         xq[dc][:],
                                         start=(dc == 0), stop=(dc == DC - 1))
                    nc.vector.tensor_copy(qt_sb[:, po * 512:(po + 1) * 512], ps[:])

            # ---------------- phase 2: pull gathered K/V into SBUF ----------
            for r in range(NC_N):
                eng = nc.sync if r % 2 == 0 else nc.scalar
                eng.dma_start(
                    ktfa[r][:].rearrange("p (a j) -> p a j", a=DC),
                    kta_out[r].rearrange("(a p) j -> p a j", p=128))
            for r in range(NC_N):
                eng = nc.sync if r % 2 == 0 else nc.scalar
                eng.dma_start(
                    ktfb[r][:].rearrange("p (a j) -> p a j", a=DC),
                    ktb_out[r].rearrange("(a p) j -> p a j", p=128))
            for r in range(NC_N):
                for dh in range(2):
                    eng = nc.sync if (2 * r + dh) % 2 == 0 else nc.scalar
                    eng.dma_start(
                        vfA[dh][r][:].rearrange("p (a j) -> p a j", a=2),
                        va_out[r].rearrange("(a p) (b j) -> p a b j",
                                            p=128, b=2)[:, :, dh, :])
            for r in range(NC_N):
                for dh in range(2):
                    eng = nc.sync if (2 * r + dh) % 2 == 0 else nc.scalar
                    eng.dma_start(
                        vfB[dh][r][:].rearrange("p (a j) -> p a j", a=2),
                        vb_out[r].rearrange("(a p) (b j) -> p a b j",
                                            p=128, b=2)[:, :, dh, :])

            # ---------------- phase 3a: S^T + exp -> P^T ----------------
            # A-half keys (kci 0,1 of every shard) first: they arrive one AG
            # earlier.  Chunk order is free -- all indexing is kc-absolute.
            with tc.tile_pool(name="spsum", bufs=6, space="PSUM") as spsum:
                kcs = [kc for kc in range(NKC) if kc % 4 < 2] + \
                      [kc for kc in range(NKC) if kc % 4 >= 2]
                for kc in kcs:
                    qt0 = kc // 8
                    qoff = 16 * kc
                    w = 512 - qoff
                    r, kci = kc // 4, kc % 4
                    ktf = ktfa if kci < 2 else ktfb
                    kcj = kci % 2
                    if qoff > 128 * qt0:
                        nc.vector.memset(
                            ptall[:, kc * 512 + 128 * qt0:kc * 512 + qoff], 0.0)
                    psT = spsum.tile([128, 512], fp32, tag="s")
                    for dc in range(DC):
                        nc.tensor.matmul(
                            psT[:, :w],
                            ktf[r][:, dc * 256 + kcj * 128:
                                   dc * 256 + (kcj + 1) * 128],
                            qt_sb[:, dc * 512 + qoff:(dc + 1) * 512],
                            start=(dc == 0), stop=(dc == DC - 1))
                    nc.vector.tensor_add(psT[:, 0:16], psT[:, 0:16],
                                         mask_sb[:])
                    nc.scalar.activation(
                        ptall[:, kc * 512 + qoff:(kc + 1) * 512], psT[:, :w],
                        mybir.ActivationFunctionType.Exp,
                        bias=0.0, scale=float(inv_sqrt_d))

            # ---------------- phase 3b: P@V + denominators ----------------
            with (
                tc.tile_pool(name="obuf", bufs=2) as opool,
                tc.tile_pool(name="opsum", bufs=2, space="PSUM") as opsum,
            ):
                for qt in range(NQT):
                    nkc = 8 * (qt + 1)
                    # A-half V rows first: they arrive one AllGather earlier.
                    qkcs = [kc for kc in range(nkc) if kc % 4 < 2] + \
                           [kc for kc in range(nkc) if kc % 4 >= 2]
                    pso = [opsum.tile([128, 512], fp32, tag=f"po{dh}",
                                      name=f"pso{qt}_{dh}") for dh in range(2)]
                    denp = opsum.tile([128, 1], fp32, tag="den",
                                      name=f"den{qt}")
                    for idx, kc in enumerate(qkcs):
                        r, sl = kc // 4, kc % 4
                        vh = vfA if sl < 2 else vfB
                        slv = sl % 2
                        lhs = ptall[:, kc * 512 + qt * 128:
                                    kc * 512 + (qt + 1) * 128]
                        st = (idx == 0)
                        sp = (idx == nkc - 1)
                        # lhsT (P^T chunk) shared by the three rhs -> one
                        # stationary load serves d-half 0, d-half 1, denom
                        nc.tensor.matmul(pso[0][:], lhs,
                                         vh[0][r][:, slv * 512:(slv + 1) * 512],
                                         start=st, stop=sp)
                        nc.tensor.matmul(pso[1][:], lhs,
                                         vh[1][r][:, slv * 512:(slv + 1) * 512],
                                         start=st, stop=sp)
                        nc.tensor.matmul(denp[:], lhs, ones_sb[:],
                                         start=st, stop=sp)
                    recip = stpool.tile([128, 1], fp32, tag="recip")
                    nc.vector.reciprocal(recip[:], denp[:])
                    o_sb = opool.tile([128, D], fp32, tag="o")
                    for dh in range(2):
                        nc.vector.tensor_scalar_mul(
                            o_sb[:, dh * 512:(dh + 1) * 512], pso[dh][:],
                            recip[:])
                    nc.sync.dma_start(out[qt * 128:(qt + 1) * 128, :], o_sb[:])

    nc.compile()
    return nc


def _get_nc():
    if "nc" not in _CACHE:
        _CACHE["nc"] = _build()
    return _CACHE["nc"]


def make_in_maps(x, Wq, Wk, Wv):
    x_bf = np.ascontiguousarray(x).astype(BF16)
    wqt = np.ascontiguousarray(Wq.astype(BF16).T)
    wkt = np.ascontiguousarray(Wk.astype(BF16).T)
    wvt = np.ascontiguousarray(Wv.astype(BF16).T)
    in_maps = []
    j = np.arange(128)[:, None]                     # key-within-chunk
    u = np.arange(16)[None, :]                      # q - 16*kc
    for c in range(NC_N):
        # q row (global) = c + 8*(16*kc + u); key = 128*kc + j
        # keep iff c + 8*u >= j  (kc-independent)
        maskT = np.where(c + 8 * u >= j, 0.0, -30000.0).astype(np.float32)
        xt_q = np.ascontiguousarray(x_bf[c::NC_N].T)
        xt_kv = np.ascontiguousarray(x_bf[c * KVROWS:(c + 1) * KVROWS].T)
        in_maps.append({"xt_q": xt_q, "xt_kv": xt_kv, "wqt": wqt,
                        "wkt": wkt, "wvt": wvt, "maskT": maskT})
    return in_maps


def run(in_maps, trace=False, tmpdir=None, trace_cores=None):
    from concourse.bass_utils import run_bass_kernel_spmd
    nc = _get_nc()
    return run_bass_kernel_spmd(nc, in_maps, core_ids=list(range(NC_N)),
                                trace=trace, tmpdir=tmpdir,
                                trace_cores=trace_cores)


def kernel(x, Wq, Wk, Wv):
    res = run(make_in_maps(np.asarray(x), np.asarray(Wq),
                           np.asarray(Wk), np.asarray(Wv)))
    full = np.empty((S, D), np.float32)
    for c in range(NC_N):
        full[c::NC_N] = res.results[c]["out"]
    return full
